# revision 1
# baseline (speedup 1.0000x reference)
"""Cross-encoding kernel for Trainium2 (Bass/Tile), 8-core batch-parallel.

Per batch b:
    query = Q W1 + b1 ; key = A W2 + b2
    S = query key^T / sqrt(d)
    eq = softmax_rows(S) @ A          (qk attention)
    ea = softmax_cols(S)^T @ Q        (kq attention)

Strategy: data-parallel over batch (16 batches -> 8 cores x 2). The two
projections are folded on the host: S = (Q M) A^T / sqrt(d) + u 1^T + 1 v^T
with M = W1 W2^T, u = Q W1 b2 / sqrt(d), v = A W2 b1 / sqrt(d) (the constant
b1.b2 term cancels in both softmaxes). Only ONE on-device projection remains
(qm = M^T Q^T) and the key side streams the raw A^T input. The rank-1 bias
terms fold into the exp's per-partition bias for free (each softmax direction
only needs the bias that does not cancel in its own normalizer).

Scores are computed in both orientations (S and S^T) on the PE so the
attention-weight matrices are always consumed as matmul lhsT in natural
layout — no on-device transposes. Softmax skips max-subtraction (|S| < ~3
for these inputs). Each orientation pass self-normalizes: denominators are
cross-partition sums of the exp'd tiles via a cheap ones-row matmul
(N=256, trivial weight load) accumulated across chunks, then fanned out to
per-partition layout with a K=1 matmul. All matmuls run in float32r.
"""
import math

import numpy as np

B, LQ, LA, D = 16, 2048, 2048, 1024
NCORES = 8
BPC = B // NCORES

_cached = {}


def _build(lq=LQ, la=LA, d=D, bpc=BPC):
    import concourse.bass as bass
    import concourse.tile as tile
    from concourse import bacc, mybir

    f32 = mybir.dt.float32
    f32r = mybir.dt.float32r
    ec_n = d // 128
    nqt, nat = lq // 128, la // 128
    nqg, nag = lq // 256, la // 256
    net = d // 128
    nqs, nas = lq // 512, la // 512
    ndh = d // 512
    inv_sqrt_d = 1.0 / math.sqrt(d)

    nc = bacc.Bacc("TRN2", target_bir_lowering=False, debug=False)

    qt_in = nc.dram_tensor("qt_in", [bpc, ec_n, 128, lq], f32r, kind="ExternalInput").ap()
    at_in = nc.dram_tensor("at_in", [bpc, ec_n, 128, la], f32r, kind="ExternalInput").ap()
    qn_in = nc.dram_tensor("qn_in", [bpc, nqt, 128, d], f32r, kind="ExternalInput").ap()
    an_in = nc.dram_tensor("an_in", [bpc, nat, 128, d], f32r, kind="ExternalInput").ap()
    # M et-major: m[et, ec, p, f] = M[ec*128+p, et*128+f]
    m_in = nc.dram_tensor("m_in", [net, ec_n, 128, 128], f32r, kind="ExternalInput").ap()
    ub_in = nc.dram_tensor("ub_in", [bpc, lq], f32, kind="ExternalInput").ap()
    vb_in = nc.dram_tensor("vb_in", [bpc, la], f32, kind="ExternalInput").ap()
    eq_out = nc.dram_tensor("eq_out", [bpc, nqt, 128, d], f32, kind="ExternalOutput").ap()
    ea_out = nc.dram_tensor("ea_out", [bpc, nat, 128, d], f32, kind="ExternalOutput").ap()

    Exp = mybir.ActivationFunctionType.Exp

    with tile.TileContext(nc) as tc:
        with (
            tc.tile_pool(name="big", bufs=1) as big,
            tc.tile_pool(name="wp", bufs=1) as wp,
            tc.tile_pool(name="streams", bufs=2) as streams,
            tc.tile_pool(name="stage", bufs=3) as stage,
            tc.tile_pool(name="ep", bufs=6) as ep,
            tc.tile_pool(name="small", bufs=1) as small,
            tc.tile_pool(name="dram", bufs=1, space=bass.MemorySpace.DRAM) as dpool,
            tc.tile_pool(name="psO", bufs=2, space=bass.MemorySpace.PSUM) as psO,
            tc.tile_pool(name="psS", bufs=2, space=bass.MemorySpace.PSUM) as psS,
            tc.tile_pool(name="psC", bufs=2, space=bass.MemorySpace.PSUM) as psC,
        ):
            ones_f32 = small.tile([128, 2], f32, tag="ones32")
            nc.vector.memset(ones_f32, 1.0)
            ones_sb = small.tile([128, 2], f32r, tag="ones")
            nc.vector.tensor_copy(out=ones_sb, in_=ones_f32)

            def projection(xt_dram_b, w_dram, out_dram, nseg):
                """out[e, s] = M^T @ X^T, qs-major; copies alternate DVE/ACT."""
                xt_full = big.tile([128, ec_n, nseg * 512], f32r, tag="X", name="xt_full")
                for blk in range(nseg):
                    nc.sync.dma_start(
                        out=xt_full[:, :, blk * 512:(blk + 1) * 512],
                        in_=xt_dram_b[:, :, blk * 512:(blk + 1) * 512]
                        .rearrange("c p q -> p c q"))
                w_sb = wp.tile([128, ec_n, d], f32r, tag="w", name="w_sb")
                for et in range(net):
                    nc.sync.dma_start(
                        out=w_sb[:, :, et * 128:(et + 1) * 128],
                        in_=w_dram[et].rearrange("c p f -> p c f"))
                k = 0
                pj = None
                for qs in range(nseg):
                    for et in range(net):
                        if k % 2 == 0:
                            pj = psO.tile([128, 2, 512], f32, tag="psO", name="pj")
                        for ec in range(ec_n):
                            nc.tensor.matmul(
                                pj[:, k % 2, :],
                                w_sb[:, ec, et * 128:(et + 1) * 128],
                                xt_full[:, ec, qs * 512:(qs + 1) * 512],
                                start=(ec == 0), stop=(ec == ec_n - 1))
                        dst = stage.tile([128, 512], f32r, tag="ktst", name="st")
                        if k % 2 == 0:
                            nc.vector.tensor_copy(out=dst, in_=pj[:, k % 2, :])
                        else:
                            nc.scalar.copy(out=dst, in_=pj[:, k % 2, :])
                        nc.sync.dma_start(
                            out=out_dram[et, :, qs * 512:(qs + 1) * 512], in_=dst)
                        k += 1

            def attn_path(lh_full, rh_scratch, nat_tile, n_groups, n_chunks,
                          bias_sb, out_dram_b):
                """One orientation pass over the score matrix.
                lh_full:   [128, ec_n, n_chunks*128] resident lhsT source.
                rh_scratch:[ec, 128, n_groups*256] DRAM, streamed per group.
                nat_tile:  [128, n_chunks, d] resident rhs for the AV matmul.
                bias_sb:   [128, n_chunks] per-partition exp bias.
                out_dram_b:[2*n_groups, 128, d] outputs, normalized inline."""
                for g in range(n_groups):
                    strm = streams.tile([128, ec_n, 256], f32r, tag="kqstream", name="strm")
                    nc.sync.dma_start(
                        out=strm,
                        in_=rh_scratch[:, :, g * 256:(g + 1) * 256]
                        .rearrange("c p a -> p c a"))
                    pacc = [psO.tile([128, d], f32, tag="psO", name="pacc")
                            for _ in range(2)]
                    cs_row = psC.tile([1, 256], f32, tag="psC", name="cs_row")

                    def consume(e_t, ch):
                        # AV + denominator matmuls for an exp'd chunk
                        for t2 in range(2):
                            for dh in range(ndh):
                                nc.tensor.matmul(
                                    pacc[t2][:, dh * 512:(dh + 1) * 512],
                                    e_t[:, t2 * 128:(t2 + 1) * 128],
                                    nat_tile[:, ch, dh * 512:(dh + 1) * 512],
                                    start=(ch == 0), stop=(ch == n_chunks - 1))
                        # denominator partial: ones^T @ E -> [1, 256]
                        nc.tensor.matmul(
                            cs_row, ones_sb[:, 0:1], e_t,
                            start=(ch == 0), stop=(ch == n_chunks - 1))

                    # software pipeline: emit chunk ch's AV matmuls after the
                    # scores of ch+1, so the exp (ACT) hides under the next
                    # scores burst instead of stalling the in-order PE stream
                    prev = None
                    for ch in range(n_chunks):
                        ps = psS.tile([128, 256], f32, tag="psS", name="ps")
                        for ec in range(ec_n):
                            nc.tensor.matmul(
                                ps, lh_full[:, ec, ch * 128:(ch + 1) * 128],
                                strm[:, ec, :],
                                start=(ec == 0), stop=(ec == ec_n - 1))
                        e_t = ep.tile([128, 256], f32r, tag="et", name="e_t")
                        nc.scalar.activation(
                            out=e_t, in_=ps, func=Exp, scale=inv_sqrt_d,
                            bias=bias_sb[:, ch:ch + 1])
                        if prev is not None:
                            consume(*prev)
                        prev = (e_t, ch)
                    consume(*prev)
                    # fan the [1, 256] sums out to per-partition [128, 2] via
                    # K=1 matmuls (each into its own psS slot: a matmul
                    # start=True clears its whole PSUM bank)
                    cs_sb = ep.tile([1, 256], f32r, tag="csrow", name="cs_sb")
                    nc.vector.tensor_copy(out=cs_sb, in_=cs_row)
                    for t2 in range(2):
                        fan = psS.tile([128, 256], f32, tag="psS", name="fan")
                        nc.tensor.matmul(
                            fan[:, 0:2], cs_sb[0:1, t2 * 128:(t2 + 1) * 128],
                            ones_sb[0:1, :], start=True, stop=True)
                        csr_t = ep.tile([128, 1], f32, tag="csr", name="csr_t")
                        nc.vector.reciprocal(out=csr_t, in_=fan[:, 0:1])
                        st = stage.tile([128, d], f32, tag="outst", name="st_o")
                        nc.vector.tensor_scalar_mul(out=st, in0=pacc[t2], scalar1=csr_t)
                        nc.sync.dma_start(out=out_dram_b[g * 2 + t2], in_=st)

            for bi in range(bpc):
                qm_s = dpool.tile([ec_n, 128, lq], f32r, tag=f"qm_s{bi}", name="qm_s")
                ub_sb = small.tile([128, nqt], f32, tag=f"ub{bi}", name="ub_sb")
                vb_sb = small.tile([128, nat], f32, tag=f"vb{bi}", name="vb_sb")
                nc.sync.dma_start(out=ub_sb, in_=ub_in[bi].rearrange("(t p) -> p t", p=128))
                nc.sync.dma_start(out=vb_sb, in_=vb_in[bi].rearrange("(t p) -> p t", p=128))

                # key side needs no projection: kT = A^T directly
                kt_full = big.tile([128, ec_n, la], f32r, tag="Y", name="kt_full")
                for blk in range(nas):
                    nc.sync.dma_start(
                        out=kt_full[:, :, blk * 512:(blk + 1) * 512],
                        in_=at_in[bi][:, :, blk * 512:(blk + 1) * 512]
                        .rearrange("c p a -> p c a"))

                # P1: qm = M^T Q^T -> DRAM scratch
                projection(qt_in[bi], m_in, qm_s, nqs)

                # EQ: ST-orientation [a, q], bias = v (per a)
                anat = big.tile([128, nat, d], f32r, tag="X", name="anat")
                nblk = min(8, nat)
                tb = nat // nblk
                for blk in range(nblk):
                    nc.sync.dma_start(
                        out=anat[:, blk * tb:(blk + 1) * tb, :],
                        in_=an_in[bi, blk * tb:(blk + 1) * tb].rearrange("t p d -> p t d"))
                attn_path(kt_full, qm_s, anat, nqg, nat, vb_sb, eq_out[bi])

                # EA: S-orientation [q, a], bias = u (per q)
                qm_full = big.tile([128, ec_n, lq], f32r, tag="Y", name="qm_full")
                for blk in range(nqs):
                    nc.sync.dma_start(
                        out=qm_full[:, :, blk * 512:(blk + 1) * 512],
                        in_=qm_s[:, :, blk * 512:(blk + 1) * 512]
                        .rearrange("c p q -> p c q"))
                qnat = big.tile([128, nqt, d], f32r, tag="X", name="qnat")
                nblk = min(8, nqt)
                tb = nqt // nblk
                for blk in range(nblk):
                    nc.sync.dma_start(
                        out=qnat[:, blk * tb:(blk + 1) * tb, :],
                        in_=qn_in[bi, blk * tb:(blk + 1) * tb].rearrange("t p d -> p t d"))
                attn_path(qm_full, at_in[bi], qnat, nag, nqt, ub_sb, ea_out[bi])

    nc.compile()
    return nc


def _get_nc():
    if "nc" not in _cached:
        _cached["nc"] = _build()
    return _cached["nc"]


def _pack_inputs(Qc, Ac, lq, la, d):
    ec_n = d // 128
    bpc = Qc.shape[0]
    return {
        "qt_in": np.ascontiguousarray(Qc.transpose(0, 2, 1)).reshape(bpc, ec_n, 128, lq),
        "at_in": np.ascontiguousarray(Ac.transpose(0, 2, 1)).reshape(bpc, ec_n, 128, la),
        "qn_in": np.ascontiguousarray(Qc).reshape(bpc, lq // 128, 128, d),
        "an_in": np.ascontiguousarray(Ac).reshape(bpc, la // 128, 128, d),
    }


def _fold_weights(W1, b1, W2, b2, d):
    """Host-side fold: M = W1 W2^T (fp64), and the rank-1 score bias vectors."""
    net = ec_n = d // 128
    M = (W1.astype(np.float64) @ W2.astype(np.float64).T).astype(np.float32)
    w1b2 = W1.astype(np.float64) @ b2.astype(np.float64)
    w2b1 = W2.astype(np.float64) @ b1.astype(np.float64)
    m_packed = np.ascontiguousarray(
        M.reshape(ec_n, 128, net, 128).transpose(2, 0, 1, 3))
    return M, m_packed, w1b2, w2b1


def _bias_vectors(Qc, Ac, w1b2, w2b1, d):
    inv = 1.0 / math.sqrt(d)
    ub = (Qc.astype(np.float64) @ w1b2 * inv).astype(np.float32)
    vb = (Ac.astype(np.float64) @ w2b1 * inv).astype(np.float32)
    return ub, vb


def _reference_fallback(Q, A, mask, W1, b1, W2, b2):
    NEG = np.float32(-1e9)
    eqs, eas = [], []
    for b in range(Q.shape[0]):
        query = Q[b] @ W1 + b1
        key = A[b] @ W2 + b2
        s = (query @ key.T) / np.float32(math.sqrt(Q.shape[-1]))
        s = np.where(mask[b] == 0, NEG, s).astype(np.float32)
        sq = s - s.max(axis=1, keepdims=True)
        eq_w = np.exp(sq); eq_w /= eq_w.sum(axis=1, keepdims=True)
        sa = s.T - s.T.max(axis=1, keepdims=True)
        ea_w = np.exp(sa); ea_w /= ea_w.sum(axis=1, keepdims=True)
        eqs.append(eq_w @ A[b])
        eas.append(ea_w @ Q[b])
    return np.stack(eqs), np.stack(eas)


def kernel(Q, A, mask, W1, b1, W2, b2):
    Q = np.ascontiguousarray(Q, dtype=np.float32)
    A = np.ascontiguousarray(A, dtype=np.float32)
    W1 = np.ascontiguousarray(W1, dtype=np.float32)
    W2 = np.ascontiguousarray(W2, dtype=np.float32)
    b1 = np.ascontiguousarray(b1, dtype=np.float32)
    b2 = np.ascontiguousarray(b2, dtype=np.float32)

    if not np.all(mask == 1):
        return _reference_fallback(Q, A, mask, W1, b1, W2, b2)

    from concourse import bass_utils

    nc = _get_nc()
    _, m_packed, w1b2, w2b1 = _fold_weights(W1, b1, W2, b2, D)
    in_maps = []
    for c in range(NCORES):
        sl = slice(c * BPC, (c + 1) * BPC)
        m = _pack_inputs(Q[sl], A[sl], LQ, LA, D)
        ub, vb = _bias_vectors(Q[sl], A[sl], w1b2, w2b1, D)
        m.update({"m_in": m_packed, "ub_in": ub, "vb_in": vb})
        in_maps.append(m)

    res = bass_utils.run_bass_kernel_spmd(nc, in_maps, core_ids=list(range(NCORES)))

    eq = np.empty((B, LQ, D), np.float32)
    ea = np.empty((B, LA, D), np.float32)
    for c in range(NCORES):
        out = res.results[c]
        eq[c * BPC:(c + 1) * BPC] = out["eq_out"].reshape(BPC, LQ, D)
        ea[c * BPC:(c + 1) * BPC] = out["ea_out"].reshape(BPC, LA, D)
    return eq, ea



# revision 2
# speedup vs baseline: 1.0875x; 1.0875x over previous
"""Cross-encoding kernel for Trainium2 (Bass/Tile), 8-core batch-parallel.

Per batch b:
    query = Q W1 + b1 ; key = A W2 + b2
    S = query key^T / sqrt(d)
    eq = softmax_rows(S) @ A          (qk attention)
    ea = softmax_cols(S)^T @ Q        (kq attention)

Strategy: data-parallel over batch (16 batches -> 8 cores x 2). The two
projections are folded on the host: S = (Q M) A^T / sqrt(d) + u 1^T + 1 v^T
with M = W1 W2^T, u = Q W1 b2 / sqrt(d), v = A W2 b1 / sqrt(d) (the b1.b2
constant cancels in both softmaxes). Only ONE on-device projection remains
(qm = M^T Q^T); the key side streams the raw A^T input.

Unlike the two-pass ancestor, the score matrix is computed ONCE, in the
S^T orientation [a, q]. E = exp(S^T/sqrt(d) + v_a) tiles (bf16) feed the
eq attention matmuls directly as lhsT, and are simultaneously transposed
by the DMA xbar (dma_start_transpose, 2-byte dtype) into a resident
E_full [q-part, a] copy that the ea pass consumes as lhsT - no second
score pass, no second exp pass. The per-q bias factor exp(u_q), which the
kq softmax needs (it does not cancel along its summation axis), is folded
on the host into the ea-pass V operand (Q rows scaled by exp(u_q)) and
into the ea denominator weights (lhsT = exp(u) instead of ones).

All heavy matmuls run in bf16 (PE full rate, half the SBUF/DMA of f32),
accumulating in f32 PSUM; softmax denominators and normalization stay
f32. Softmax skips max-subtraction (|scores| < ~3 for these inputs).
Denominators are cross-partition ones-matmul sums accumulated in PSUM,
fanned out to per-partition layout with K=1 matmuls (f32r path).
"""
import math

import numpy as np

B, LQ, LA, D = 16, 2048, 2048, 1024
NCORES = 8
BPC = B // NCORES

_cached = {}


def _build(lq=LQ, la=LA, d=D, bpc=BPC):
    import concourse.bass as bass
    import concourse.tile as tile
    from concourse import bacc, mybir

    f32 = mybir.dt.float32
    f32r = mybir.dt.float32r
    bf16 = mybir.dt.bfloat16
    ec_n = d // 128            # 8 contraction chunks over d
    net = d // 128             # 8 output-feature tiles in the projection
    nqt, nat = lq // 128, la // 128   # 16 q-tiles / a-tiles
    nqg = lq // 256            # 8 scores groups of 256 q
    nqs = lq // 512            # 4 projection segments
    ndh = d // 512             # 2 d-halves for AV matmuls
    inv_sqrt_d = 1.0 / math.sqrt(d)

    nc = bacc.Bacc("TRN2", target_bir_lowering=False, debug=False)

    qt_in = nc.dram_tensor("qt_in", [bpc, ec_n, 128, lq], bf16, kind="ExternalInput").ap()
    at_in = nc.dram_tensor("at_in", [bpc, ec_n, 128, la], bf16, kind="ExternalInput").ap()
    an_in = nc.dram_tensor("an_in", [bpc, nat, 128, d], bf16, kind="ExternalInput").ap()
    # qn_in rows pre-scaled by exp(u_q) on host
    qn_in = nc.dram_tensor("qn_in", [bpc, nqt, 128, d], bf16, kind="ExternalInput").ap()
    # M et-major: m[et, ec, p, f] = M[ec*128+p, et*128+f]
    m_in = nc.dram_tensor("m_in", [net, ec_n, 128, 128], bf16, kind="ExternalInput").ap()
    vb_in = nc.dram_tensor("vb_in", [bpc, la], f32, kind="ExternalInput").ap()
    eub_in = nc.dram_tensor("eub_in", [bpc, lq], bf16, kind="ExternalInput").ap()
    eq_out = nc.dram_tensor("eq_out", [bpc, nqt, 128, d], f32, kind="ExternalOutput").ap()
    ea_out = nc.dram_tensor("ea_out", [bpc, nat, 128, d], f32, kind="ExternalOutput").ap()

    Exp = mybir.ActivationFunctionType.Exp

    with tile.TileContext(nc) as tc:
        with (
            tc.tile_pool(name="big", bufs=1) as big,
            tc.tile_pool(name="wp", bufs=1) as wp,
            tc.tile_pool(name="streams", bufs=2) as streams,
            tc.tile_pool(name="stage", bufs=2) as stage,
            tc.tile_pool(name="ep", bufs=4) as ep,
            tc.tile_pool(name="small", bufs=1) as small,
            tc.tile_pool(name="dram", bufs=1, space=bass.MemorySpace.DRAM) as dpool,
            tc.tile_pool(name="psO", bufs=2, space=bass.MemorySpace.PSUM) as psO,
            tc.tile_pool(name="psS", bufs=2, space=bass.MemorySpace.PSUM) as psS,
            tc.tile_pool(name="psC", bufs=2, space=bass.MemorySpace.PSUM) as psC,
        ):
            ones_f32 = small.tile([128, 2], f32, tag="ones32")
            nc.vector.memset(ones_f32, 1.0)
            ones_r = small.tile([128, 2], f32r, tag="onesr")
            nc.vector.tensor_copy(out=ones_r, in_=ones_f32)
            ones_bf = small.tile([128, 2], bf16, tag="onesbf")
            nc.vector.tensor_copy(out=ones_bf, in_=ones_f32)

            # M resident for the whole kernel (both batches)
            w_sb = wp.tile([128, ec_n, d], bf16, tag="w", name="w_sb")
            for et in range(net):
                nc.sync.dma_start(
                    out=w_sb[:, :, et * 128:(et + 1) * 128],
                    in_=m_in[et].rearrange("c p f -> p c f"))

            for bi in range(bpc):
                vb_sb = small.tile([128, nat], f32, tag=f"vb{bi}", name="vb_sb")
                eub_sb = small.tile([128, nqt], bf16, tag=f"eub{bi}", name="eub_sb")
                nc.sync.dma_start(out=vb_sb, in_=vb_in[bi].rearrange("(t p) -> p t", p=128))
                nc.sync.dma_start(out=eub_sb, in_=eub_in[bi].rearrange("(t p) -> p t", p=128))

                # key side: A^T resident (scores lhsT)
                at_full = big.tile([128, ec_n, la], bf16, tag="at", name="at_full")
                for blk in range(la // 512):
                    nc.sync.dma_start(
                        out=at_full[:, :, blk * 512:(blk + 1) * 512],
                        in_=at_in[bi][:, :, blk * 512:(blk + 1) * 512]
                        .rearrange("c p a -> p c a"))

                # P1: qm = M^T Q^T -> DRAM scratch (bf16), qs-major so the
                # scores stream of group g only needs segment g//2 done
                qt_full = big.tile([128, ec_n, lq], bf16, tag="qtan", name="qt_full")
                for blk in range(nqs):
                    nc.sync.dma_start(
                        out=qt_full[:, :, blk * 512:(blk + 1) * 512],
                        in_=qt_in[bi][:, :, blk * 512:(blk + 1) * 512]
                        .rearrange("c p q -> p c q"))
                qm_s = dpool.tile([ec_n, 128, lq], bf16, tag=f"qm_s{bi}", name="qm_s")
                k = 0
                for qs in range(nqs):
                    for et in range(net):
                        pj = psS.tile([128, 512], f32, tag="psS", name="pj")
                        for ec in range(ec_n):
                            nc.tensor.matmul(
                                pj, w_sb[:, ec, et * 128:(et + 1) * 128],
                                qt_full[:, ec, qs * 512:(qs + 1) * 512],
                                start=(ec == 0), stop=(ec == ec_n - 1))
                        dst = stage.tile([128, 512], bf16, tag="pst", name="dst")
                        if k % 2 == 0:
                            nc.vector.tensor_copy(out=dst, in_=pj)
                        else:
                            nc.scalar.copy(out=dst, in_=pj)
                        nc.sync.dma_start(
                            out=qm_s[et, :, qs * 512:(qs + 1) * 512], in_=dst)
                        k += 1

                # A natural (eq AV rhs) reuses the qt slot; per-tile DMAs in
                # chunk order so chunk 0 is available right after proj ends
                anat = big.tile([128, nat, d], bf16, tag="qtan", name="anat")
                for ch in range(nat):
                    nc.sync.dma_start(
                        out=anat[:, ch, :],
                        in_=an_in[bi, ch].rearrange("p d -> p d"))
                # prefetch the ea-pass operand during the scores pass
                qnat = big.tile([128, nqt, d], bf16, tag="qnat", name="qnat")
                nblk = 4
                tb = nqt // nblk
                for blk in range(nblk):
                    nc.sync.dma_start(
                        out=qnat[:, blk * tb:(blk + 1) * tb, :],
                        in_=qn_in[bi, blk * tb:(blk + 1) * tb].rearrange("t p d -> p t d"))

                e_full = big.tile([128, nqt, la], bf16, tag="efull", name="e_full")

                # P2: scores (S^T orientation) + exp + eq attention + xbar
                # transpose of E tiles into e_full
                for g in range(nqg):
                    strm = streams.tile([128, ec_n, 256], bf16, tag="strm", name="strm")
                    nc.sync.dma_start(
                        out=strm,
                        in_=qm_s[:, :, g * 256:(g + 1) * 256]
                        .rearrange("c p q -> p c q"))
                    pacc = [psO.tile([128, d], f32, tag="pacc", name="pacc")
                            for _ in range(2)]
                    cs_row = psC.tile([1, 256], f32, tag="psC", name="cs_row")

                    def consume(e_t, ch, g=g, pacc=pacc, cs_row=cs_row):
                        for t2 in range(2):
                            for dh in range(ndh):
                                nc.tensor.matmul(
                                    pacc[t2][:, dh * 512:(dh + 1) * 512],
                                    e_t[:, t2 * 128:(t2 + 1) * 128],
                                    anat[:, ch, dh * 512:(dh + 1) * 512],
                                    start=(ch == 0), stop=(ch == nat - 1))
                        # eq denominator partial: ones^T @ E -> [1, 256]
                        nc.tensor.matmul(
                            cs_row, ones_bf[:, 0:1], e_t,
                            start=(ch == 0), stop=(ch == nat - 1))
                        # E^T tile -> E orientation, via DMA xbar
                        nc.scalar.dma_start_transpose(
                            out=e_full[:, 2 * g:2 * g + 2, ch * 128:(ch + 1) * 128],
                            in_=e_t)

                    # software pipeline: consume chunk ch after the scores of
                    # ch+1 are issued, so the exp hides under the PE stream
                    prev = None
                    for ch in range(nat):
                        ps = psS.tile([128, 256], f32, tag="psS", name="ps")
                        for ec in range(ec_n):
                            nc.tensor.matmul(
                                ps, at_full[:, ec, ch * 128:(ch + 1) * 128],
                                strm[:, ec, :],
                                start=(ec == 0), stop=(ec == ec_n - 1))
                        e_t = ep.tile([128, 256], bf16, tag="et", name="e_t")
                        nc.scalar.activation(
                            out=e_t, in_=ps, func=Exp, scale=inv_sqrt_d,
                            bias=vb_sb[:, ch:ch + 1])
                        if prev is not None:
                            consume(*prev)
                        prev = (e_t, ch)
                    consume(*prev)

                    # normalize + store the group's two eq tiles
                    cs_sb = ep.tile([1, 256], f32r, tag="csrow", name="cs_sb")
                    nc.vector.tensor_copy(out=cs_sb, in_=cs_row)
                    for t2 in range(2):
                        fan = psS.tile([128, 2], f32, tag="psS", name="fan")
                        nc.tensor.matmul(
                            fan, cs_sb[0:1, t2 * 128:(t2 + 1) * 128],
                            ones_r[0:1, :], start=True, stop=True)
                        csr_t = ep.tile([128, 1], f32, tag="csr", name="csr_t")
                        nc.vector.reciprocal(out=csr_t, in_=fan[:, 0:1])
                        st = stage.tile([128, d], f32, tag="outst", name="st_o")
                        nc.vector.tensor_scalar_mul(out=st, in0=pacc[t2], scalar1=csr_t)
                        nc.sync.dma_start(out=eq_out[bi, g * 2 + t2], in_=st)

                # P3: ea attention from the transposed E
                for at in range(nat):
                    pea = psO.tile([128, d], f32, tag="pacc", name="pea")
                    colps = psC.tile([1, 128], f32, tag="psC", name="colps")
                    for qch in range(nqt):
                        lhs = e_full[:, qch, at * 128:(at + 1) * 128]
                        for dh in range(ndh):
                            nc.tensor.matmul(
                                pea[:, dh * 512:(dh + 1) * 512],
                                lhs, qnat[:, qch, dh * 512:(dh + 1) * 512],
                                start=(qch == 0), stop=(qch == nqt - 1))
                        # ea denominator: exp(u)^T @ E -> [1, 128]
                        nc.tensor.matmul(
                            colps, eub_sb[:, qch:qch + 1], lhs,
                            start=(qch == 0), stop=(qch == nqt - 1))
                    cl_sb = ep.tile([1, 128], f32r, tag="clrow", name="cl_sb")
                    nc.vector.tensor_copy(out=cl_sb, in_=colps)
                    fan = psS.tile([128, 2], f32, tag="psS", name="fan2")
                    nc.tensor.matmul(
                        fan, cl_sb[0:1, :], ones_r[0:1, :], start=True, stop=True)
                    car_t = ep.tile([128, 1], f32, tag="csr", name="car_t")
                    nc.vector.reciprocal(out=car_t, in_=fan[:, 0:1])
                    st = stage.tile([128, d], f32, tag="outst", name="st_a")
                    nc.vector.tensor_scalar_mul(out=st, in0=pea, scalar1=car_t)
                    nc.sync.dma_start(out=ea_out[bi, at], in_=st)

    nc.compile()
    return nc


def _get_nc():
    if "nc" not in _cached:
        _cached["nc"] = _build()
    return _cached["nc"]


def _pack_inputs(Qc, Ac, eub, lq, la, d):
    import ml_dtypes

    bf = ml_dtypes.bfloat16
    ec_n = d // 128
    bpc = Qc.shape[0]
    qn = (Qc * eub[:, :, None]).astype(bf)
    return {
        "qt_in": np.ascontiguousarray(Qc.transpose(0, 2, 1)).astype(bf).reshape(bpc, ec_n, 128, lq),
        "at_in": np.ascontiguousarray(Ac.transpose(0, 2, 1)).astype(bf).reshape(bpc, ec_n, 128, la),
        "qn_in": np.ascontiguousarray(qn).reshape(bpc, lq // 128, 128, d),
        "an_in": Ac.astype(bf).reshape(bpc, la // 128, 128, d),
        "eub_in": eub.astype(bf),
    }


def _fold_weights(W1, b1, W2, b2, d):
    """Host-side fold: M = W1 W2^T (fp64), and the rank-1 score bias vectors."""
    import ml_dtypes

    net = ec_n = d // 128
    M = (W1.astype(np.float64) @ W2.astype(np.float64).T).astype(np.float32)
    w1b2 = W1.astype(np.float64) @ b2.astype(np.float64)
    w2b1 = W2.astype(np.float64) @ b1.astype(np.float64)
    m_packed = np.ascontiguousarray(
        M.reshape(ec_n, 128, net, 128).transpose(2, 0, 1, 3)).astype(ml_dtypes.bfloat16)
    return M, m_packed, w1b2, w2b1


def _bias_vectors(Qc, Ac, w1b2, w2b1, d):
    inv = 1.0 / math.sqrt(d)
    ub = (Qc.astype(np.float64) @ w1b2 * inv).astype(np.float32)
    vb = (Ac.astype(np.float64) @ w2b1 * inv).astype(np.float32)
    return ub, vb


def _reference_fallback(Q, A, mask, W1, b1, W2, b2):
    NEG = np.float32(-1e9)
    eqs, eas = [], []
    for b in range(Q.shape[0]):
        query = Q[b] @ W1 + b1
        key = A[b] @ W2 + b2
        s = (query @ key.T) / np.float32(math.sqrt(Q.shape[-1]))
        s = np.where(mask[b] == 0, NEG, s).astype(np.float32)
        sq = s - s.max(axis=1, keepdims=True)
        eq_w = np.exp(sq); eq_w /= eq_w.sum(axis=1, keepdims=True)
        sa = s.T - s.T.max(axis=1, keepdims=True)
        ea_w = np.exp(sa); ea_w /= ea_w.sum(axis=1, keepdims=True)
        eqs.append(eq_w @ A[b])
        eas.append(ea_w @ Q[b])
    return np.stack(eqs), np.stack(eas)


def kernel(Q, A, mask, W1, b1, W2, b2):
    Q = np.ascontiguousarray(Q, dtype=np.float32)
    A = np.ascontiguousarray(A, dtype=np.float32)
    W1 = np.ascontiguousarray(W1, dtype=np.float32)
    W2 = np.ascontiguousarray(W2, dtype=np.float32)
    b1 = np.ascontiguousarray(b1, dtype=np.float32)
    b2 = np.ascontiguousarray(b2, dtype=np.float32)

    if not np.all(mask == 1):
        return _reference_fallback(Q, A, mask, W1, b1, W2, b2)

    from concourse import bass_utils

    nc = _get_nc()
    _, m_packed, w1b2, w2b1 = _fold_weights(W1, b1, W2, b2, D)
    in_maps = []
    for c in range(NCORES):
        sl = slice(c * BPC, (c + 1) * BPC)
        ub, vb = _bias_vectors(Q[sl], A[sl], w1b2, w2b1, D)
        m = _pack_inputs(Q[sl], A[sl], np.exp(ub), LQ, LA, D)
        m.update({"m_in": m_packed, "vb_in": vb})
        in_maps.append(m)

    res = bass_utils.run_bass_kernel_spmd(nc, in_maps, core_ids=list(range(NCORES)))

    eq = np.empty((B, LQ, D), np.float32)
    ea = np.empty((B, LA, D), np.float32)
    for c in range(NCORES):
        out = res.results[c]
        eq[c * BPC:(c + 1) * BPC] = out["eq_out"].reshape(BPC, LQ, D)
        ea[c * BPC:(c + 1) * BPC] = out["ea_out"].reshape(BPC, LA, D)
    return eq, ea


# revision 6
# speedup vs baseline: 1.2010x; 1.1044x over previous
"""Cross-encoding kernel for Trainium2 (Bass/Tile), 8-core batch-parallel.

Per batch b:
    query = Q W1 + b1 ; key = A W2 + b2
    S = query key^T / sqrt(d)
    eq = softmax_rows(S) @ A          (qk attention)
    ea = softmax_cols(S)^T @ Q        (kq attention)

Strategy: data-parallel over batch (16 batches -> 8 cores x 2). The two
projections are folded on the host: S = (Q M) A^T / sqrt(d) + u 1^T + 1 v^T
with M = W1 W2^T, u = Q W1 b2 / sqrt(d), v = A W2 b1 / sqrt(d) (the b1.b2
constant cancels in both softmaxes). Only ONE on-device projection remains
(qm = M^T Q^T); the key side streams the raw A^T input.

Unlike the two-pass ancestor, the score matrix is computed ONCE, in the
S^T orientation [a, q]. E = exp(S^T/sqrt(d) + v_a) tiles (bf16) feed the
eq attention matmuls directly as lhsT, and are simultaneously transposed
by the DMA xbar (dma_start_transpose, 2-byte dtype) into a resident
E_full [q-part, a] copy that the ea pass consumes as lhsT - no second
score pass, no second exp pass. The per-q bias factor exp(u_q), which the
kq softmax needs (it does not cancel along its summation axis), is folded
on the host into the ea-pass V operand (Q rows scaled by exp(u_q)) and
into the ea denominator weights (lhsT = exp(u) instead of ones).

All heavy matmuls run in bf16 (PE full rate, half the SBUF/DMA of f32),
accumulating in f32 PSUM; softmax denominators and normalization stay
f32. Softmax skips max-subtraction (|scores| < ~3 for these inputs).
Denominators are cross-partition ones-matmul sums accumulated in PSUM,
fanned out to per-partition layout with K=1 matmuls (f32r path).
"""
import math

import numpy as np

B, LQ, LA, D = 16, 2048, 2048, 1024
NCORES = 8
BPC = B // NCORES

_cached = {}


def _build(lq=LQ, la=LA, d=D, bpc=BPC):
    import concourse.bass as bass
    import concourse.tile as tile
    from concourse import bacc, mybir

    f32 = mybir.dt.float32
    f32r = mybir.dt.float32r
    bf16 = mybir.dt.bfloat16
    ec_n = d // 128            # 8 contraction chunks over d
    net = d // 128             # 8 output-feature tiles in the projection
    nqt, nat = lq // 128, la // 128   # 16 q-tiles / a-tiles
    nqg = lq // 256            # 8 scores groups of 256 q
    nqs = lq // 512            # 4 projection segments
    ndh = d // 512             # 2 d-halves for AV matmuls
    inv_sqrt_d = 1.0 / math.sqrt(d)

    nc = bacc.Bacc("TRN2", target_bir_lowering=False, debug=False)

    qt_in = nc.dram_tensor("qt_in", [bpc, ec_n, 128, lq], bf16, kind="ExternalInput").ap()
    at_in = nc.dram_tensor("at_in", [bpc, ec_n, 128, la], bf16, kind="ExternalInput").ap()
    an_in = nc.dram_tensor("an_in", [bpc, nat, 128, d], bf16, kind="ExternalInput").ap()
    # qn_in rows pre-scaled by exp(u_q) on host
    qn_in = nc.dram_tensor("qn_in", [bpc, nqt, 128, d], bf16, kind="ExternalInput").ap()
    # M et-major: m[et, ec, p, f] = M[ec*128+p, et*128+f]
    m_in = nc.dram_tensor("m_in", [net, ec_n, 128, 128], bf16, kind="ExternalInput").ap()
    vb_in = nc.dram_tensor("vb_in", [bpc, la], f32, kind="ExternalInput").ap()
    eub_in = nc.dram_tensor("eub_in", [bpc, lq], bf16, kind="ExternalInput").ap()
    eq_out = nc.dram_tensor("eq_out", [bpc, nqt, 128, d], f32, kind="ExternalOutput").ap()
    ea_out = nc.dram_tensor("ea_out", [bpc, nat, 128, d], f32, kind="ExternalOutput").ap()

    Exp = mybir.ActivationFunctionType.Exp

    with tile.TileContext(nc) as tc:
        with (
            tc.tile_pool(name="big", bufs=1) as big,
            tc.tile_pool(name="wp", bufs=1) as wp,
            tc.tile_pool(name="streams", bufs=2) as streams,
            tc.tile_pool(name="stage", bufs=2) as stage,
            tc.tile_pool(name="ep", bufs=4) as ep,
            tc.tile_pool(name="small", bufs=1) as small,
            tc.tile_pool(name="dram", bufs=1, space=bass.MemorySpace.DRAM) as dpool,
            tc.tile_pool(name="psO", bufs=2, space=bass.MemorySpace.PSUM) as psO,
            tc.tile_pool(name="psS", bufs=2, space=bass.MemorySpace.PSUM) as psS,
            tc.tile_pool(name="psC", bufs=2, space=bass.MemorySpace.PSUM) as psC,
        ):
            ones_f32 = small.tile([128, 2], f32, tag="ones32")
            nc.vector.memset(ones_f32, 1.0)
            ones_r = small.tile([128, 2], f32r, tag="onesr")
            nc.vector.tensor_copy(out=ones_r, in_=ones_f32)
            ones_bf = small.tile([128, 2], bf16, tag="onesbf")
            nc.vector.tensor_copy(out=ones_bf, in_=ones_f32)

            # M resident for the whole kernel (both batches)
            w_sb = wp.tile([128, ec_n, d], bf16, tag="w", name="w_sb")
            for et in range(net):
                nc.sync.dma_start(
                    out=w_sb[:, :, et * 128:(et + 1) * 128],
                    in_=m_in[et].rearrange("c p f -> p c f"))

            # Deferred normalization: the fan matmuls + reciprocal + PSUM
            # eviction of a finished accumulator are NOT emitted at its
            # group's end (the PE queue is in-order: a fan waiting on the
            # DVE copy of the denominator row stalls every matmul behind
            # it). Instead the cross-partition row is copied to SBUF right
            # when it completes, and the rest is flushed 1-2 chunks into
            # the NEXT group/tile, by which time the copy has landed.
            pending = [None]

            def flush_pending():
                if pending[0] is None:
                    return
                row_sb, accs, outs = pending[0]
                pending[0] = None
                for j, (acc, out_dram) in enumerate(zip(accs, outs)):
                    fan = psS.tile([128, 2], f32, tag="psS", name="fan")
                    nc.tensor.matmul(
                        fan, row_sb[0:1, j * 128:(j + 1) * 128],
                        ones_r[0:1, :], start=True, stop=True)
                    csr_t = ep.tile([128, 1], f32, tag="csr", name="csr_t")
                    nc.vector.reciprocal(out=csr_t, in_=fan[:, 0:1])
                    st = stage.tile([128, d], f32, tag="outst", name="st_o")
                    nc.vector.tensor_scalar_mul(
                        out=st[:, 0:512], in0=acc[:, 0:512], scalar1=csr_t)
                    nc.scalar.activation(
                        out=st[:, 512:1024], in_=acc[:, 512:1024],
                        func=mybir.ActivationFunctionType.Copy, scale=csr_t)
                    nc.sync.dma_start(out=out_dram, in_=st)

            for bi in range(bpc):
                vb_sb = small.tile([128, nat], f32, tag=f"vb{bi}", name="vb_sb")
                eub_sb = small.tile([128, nqt], bf16, tag=f"eub{bi}", name="eub_sb")
                nc.sync.dma_start(out=vb_sb, in_=vb_in[bi].rearrange("(t p) -> p t", p=128))
                nc.sync.dma_start(out=eub_sb, in_=eub_in[bi].rearrange("(t p) -> p t", p=128))

                # P1 operand first (proj is the first PE consumer)
                qt_full = big.tile([128, ec_n, lq], bf16, tag="qtan", name="qt_full")
                for blk in range(nqs):
                    nc.sync.dma_start(
                        out=qt_full[:, :, blk * 512:(blk + 1) * 512],
                        in_=qt_in[bi][:, :, blk * 512:(blk + 1) * 512]
                        .rearrange("c p q -> p c q"))
                # key side: A^T resident (scores lhsT)
                at_full = big.tile([128, ec_n, la], bf16, tag="at", name="at_full")
                for blk in range(la // 512):
                    nc.sync.dma_start(
                        out=at_full[:, :, blk * 512:(blk + 1) * 512],
                        in_=at_in[bi][:, :, blk * 512:(blk + 1) * 512]
                        .rearrange("c p a -> p c a"))

                # P2 stream tiles, loaded with 2-group lookahead
                strm_tiles = {}
                qm_s = dpool.tile([ec_n, 128, lq], bf16, tag=f"qm_s{bi}", name="qm_s")

                def load_strm(g, qm_s=qm_s, strm_tiles=strm_tiles):
                    t = streams.tile([128, ec_n, 256], bf16, tag="strm", name="strm")
                    nc.sync.dma_start(
                        out=t,
                        in_=qm_s[:, :, g * 256:(g + 1) * 256]
                        .rearrange("c p q -> p c q"))
                    strm_tiles[g] = t

                # P1: qm = M^T Q^T -> DRAM scratch (bf16), qs-major so the
                # scores stream of group g only needs segment g//2 done
                k = 0
                for qs in range(nqs):
                    for et in range(net):
                        pj = psS.tile([128, 512], f32, tag="psS", name="pj")
                        for ec in range(ec_n):
                            nc.tensor.matmul(
                                pj, w_sb[:, ec, et * 128:(et + 1) * 128],
                                qt_full[:, ec, qs * 512:(qs + 1) * 512],
                                start=(ec == 0), stop=(ec == ec_n - 1))
                        dst = stage.tile([128, 512], bf16, tag="pst", name="dst")
                        if k % 2 == 0:
                            nc.vector.tensor_copy(out=dst, in_=pj)
                        else:
                            nc.scalar.copy(out=dst, in_=pj)
                        nc.sync.dma_start(
                            out=qm_s[et, :, qs * 512:(qs + 1) * 512], in_=dst)
                        k += 1
                    if qs == 0:
                        load_strm(0)
                        load_strm(1)

                # A natural (eq AV rhs) reuses the qt slot; per-tile DMAs in
                # chunk order so chunk 0 is available right after proj ends
                anat = big.tile([128, nat, d], bf16, tag="qtan", name="anat")
                for ch in range(nat):
                    nc.sync.dma_start(
                        out=anat[:, ch, :],
                        in_=an_in[bi, ch].rearrange("p d -> p d"))
                # prefetch the ea-pass operand during the scores pass
                qnat = big.tile([128, nqt, d], bf16, tag="qnat", name="qnat")
                nblk = 4
                tb = nqt // nblk
                for blk in range(nblk):
                    nc.sync.dma_start(
                        out=qnat[:, blk * tb:(blk + 1) * tb, :],
                        in_=qn_in[bi, blk * tb:(blk + 1) * tb].rearrange("t p d -> p t d"))

                e_full = big.tile([128, nqt, la], bf16, tag="efull", name="e_full")

                # P2: scores (S^T orientation) + exp + eq attention + xbar
                # transpose of E tiles into e_full
                for g in range(nqg):
                    strm = strm_tiles.pop(g)
                    pacc = [psO.tile([128, d], f32, tag="pacc", name="pacc")
                            for _ in range(2)]
                    cs_row = psC.tile([1, 256], f32, tag="psC", name="cs_row")

                    def consume(e_t, ch, g=g, pacc=pacc, cs_row=cs_row):
                        for t2 in range(2):
                            for dh in range(ndh):
                                nc.tensor.matmul(
                                    pacc[t2][:, dh * 512:(dh + 1) * 512],
                                    e_t[:, t2 * 128:(t2 + 1) * 128],
                                    anat[:, ch, dh * 512:(dh + 1) * 512],
                                    start=(ch == 0), stop=(ch == nat - 1))
                        # eq denominator partial: ones^T @ E -> [1, 256]
                        nc.tensor.matmul(
                            cs_row, ones_bf[:, 0:1], e_t,
                            start=(ch == 0), stop=(ch == nat - 1))
                        # E^T tile -> E orientation, via DMA xbar; alternate
                        # the issuing HWDGE queue (the ucode instruction
                        # occupies the issuing engine for ~1.2us)
                        eng = nc.sync if ch % 2 == 0 else nc.scalar
                        eng.dma_start_transpose(
                            out=e_full[:, 2 * g:2 * g + 2, ch * 128:(ch + 1) * 128],
                            in_=e_t)

                    # software pipeline, depth 2: consume chunk ch after the
                    # scores+exp of ch+2, so exp latency and the deferred
                    # norm flush both hide under the PE stream
                    from collections import deque
                    q = deque()
                    for ch in range(nat):
                        ps = psS.tile([128, 256], f32, tag="psS", name="ps")
                        for ec in range(ec_n):
                            nc.tensor.matmul(
                                ps, at_full[:, ec, ch * 128:(ch + 1) * 128],
                                strm[:, ec, :],
                                start=(ec == 0), stop=(ec == ec_n - 1))
                        e_t = ep.tile([128, 256], bf16, tag="et", name="e_t")
                        nc.scalar.activation(
                            out=e_t, in_=ps, func=Exp, scale=inv_sqrt_d,
                            bias=vb_sb[:, ch:ch + 1])
                        if ch == 1:
                            flush_pending()
                            if g + 2 < nqg:
                                load_strm(g + 2)
                        q.append((e_t, ch))
                        if len(q) > 2:
                            consume(*q.popleft())
                    while q:
                        consume(*q.popleft())

                    # denominator row -> SBUF now; fans/eviction deferred
                    cs_sb = ep.tile([1, 256], f32r, tag="csrow", name="cs_sb")
                    nc.vector.tensor_copy(out=cs_sb, in_=cs_row)
                    pending[0] = (cs_sb, pacc,
                                  [eq_out[bi, g * 2], eq_out[bi, g * 2 + 1]])

                # P3: ea attention from the transposed E
                for at in range(nat):
                    if at == 0:
                        # last eq group's accumulators must evict before
                        # pea(0) can claim its PSUM slot
                        flush_pending()
                    pea = psO.tile([128, d], f32, tag="pacc", name="pea")
                    colps = psC.tile([1, 128], f32, tag="psC", name="colps")
                    for qch in range(nqt):
                        lhs = e_full[:, qch, at * 128:(at + 1) * 128]
                        for dh in range(ndh):
                            nc.tensor.matmul(
                                pea[:, dh * 512:(dh + 1) * 512],
                                lhs, qnat[:, qch, dh * 512:(dh + 1) * 512],
                                start=(qch == 0), stop=(qch == nqt - 1))
                        # ea denominator: exp(u)^T @ E -> [1, 128]
                        nc.tensor.matmul(
                            colps, eub_sb[:, qch:qch + 1], lhs,
                            start=(qch == 0), stop=(qch == nqt - 1))
                        if qch == 1 and at > 0:
                            flush_pending()
                    cl_sb = ep.tile([1, 128], f32r, tag="csrow", name="cl_sb")
                    nc.vector.tensor_copy(out=cl_sb, in_=colps)
                    pending[0] = (cl_sb, [pea], [ea_out[bi, at]])

            flush_pending()

    nc.compile()
    return nc


def _get_nc():
    if "nc" not in _cached:
        _cached["nc"] = _build()
    return _cached["nc"]


def _pack_inputs(Qc, Ac, eub, lq, la, d):
    import ml_dtypes

    bf = ml_dtypes.bfloat16
    ec_n = d // 128
    bpc = Qc.shape[0]
    qn = (Qc * eub[:, :, None]).astype(bf)
    return {
        "qt_in": np.ascontiguousarray(Qc.transpose(0, 2, 1)).astype(bf).reshape(bpc, ec_n, 128, lq),
        "at_in": np.ascontiguousarray(Ac.transpose(0, 2, 1)).astype(bf).reshape(bpc, ec_n, 128, la),
        "qn_in": np.ascontiguousarray(qn).reshape(bpc, lq // 128, 128, d),
        "an_in": Ac.astype(bf).reshape(bpc, la // 128, 128, d),
        "eub_in": eub.astype(bf),
    }


def _fold_weights(W1, b1, W2, b2, d):
    """Host-side fold: M = W1 W2^T (fp64), and the rank-1 score bias vectors."""
    import ml_dtypes

    net = ec_n = d // 128
    M = (W1.astype(np.float64) @ W2.astype(np.float64).T).astype(np.float32)
    w1b2 = W1.astype(np.float64) @ b2.astype(np.float64)
    w2b1 = W2.astype(np.float64) @ b1.astype(np.float64)
    m_packed = np.ascontiguousarray(
        M.reshape(ec_n, 128, net, 128).transpose(2, 0, 1, 3)).astype(ml_dtypes.bfloat16)
    return M, m_packed, w1b2, w2b1


def _bias_vectors(Qc, Ac, w1b2, w2b1, d):
    inv = 1.0 / math.sqrt(d)
    ub = (Qc.astype(np.float64) @ w1b2 * inv).astype(np.float32)
    vb = (Ac.astype(np.float64) @ w2b1 * inv).astype(np.float32)
    return ub, vb


def _reference_fallback(Q, A, mask, W1, b1, W2, b2):
    NEG = np.float32(-1e9)
    eqs, eas = [], []
    for b in range(Q.shape[0]):
        query = Q[b] @ W1 + b1
        key = A[b] @ W2 + b2
        s = (query @ key.T) / np.float32(math.sqrt(Q.shape[-1]))
        s = np.where(mask[b] == 0, NEG, s).astype(np.float32)
        sq = s - s.max(axis=1, keepdims=True)
        eq_w = np.exp(sq); eq_w /= eq_w.sum(axis=1, keepdims=True)
        sa = s.T - s.T.max(axis=1, keepdims=True)
        ea_w = np.exp(sa); ea_w /= ea_w.sum(axis=1, keepdims=True)
        eqs.append(eq_w @ A[b])
        eas.append(ea_w @ Q[b])
    return np.stack(eqs), np.stack(eas)


def kernel(Q, A, mask, W1, b1, W2, b2):
    Q = np.ascontiguousarray(Q, dtype=np.float32)
    A = np.ascontiguousarray(A, dtype=np.float32)
    W1 = np.ascontiguousarray(W1, dtype=np.float32)
    W2 = np.ascontiguousarray(W2, dtype=np.float32)
    b1 = np.ascontiguousarray(b1, dtype=np.float32)
    b2 = np.ascontiguousarray(b2, dtype=np.float32)

    if not np.all(mask == 1):
        return _reference_fallback(Q, A, mask, W1, b1, W2, b2)

    from concourse import bass_utils

    nc = _get_nc()
    _, m_packed, w1b2, w2b1 = _fold_weights(W1, b1, W2, b2, D)
    in_maps = []
    for c in range(NCORES):
        sl = slice(c * BPC, (c + 1) * BPC)
        ub, vb = _bias_vectors(Q[sl], A[sl], w1b2, w2b1, D)
        m = _pack_inputs(Q[sl], A[sl], np.exp(ub), LQ, LA, D)
        m.update({"m_in": m_packed, "vb_in": vb})
        in_maps.append(m)

    res = bass_utils.run_bass_kernel_spmd(nc, in_maps, core_ids=list(range(NCORES)))

    eq = np.empty((B, LQ, D), np.float32)
    ea = np.empty((B, LA, D), np.float32)
    for c in range(NCORES):
        out = res.results[c]
        eq[c * BPC:(c + 1) * BPC] = out["eq_out"].reshape(BPC, LQ, D)
        ea[c * BPC:(c + 1) * BPC] = out["ea_out"].reshape(BPC, LA, D)
    return eq, ea
